# revision 10
# baseline (speedup 1.0000x reference)
"""AdaptiveGridKANLayer on 8 TRN2 NeuronCores.

out[b,o] = sum_i sum_g exp(-((x[b,i]-c_g)/w)^2) * coeffs[o,i,g]
         + sum_i silu(x[b,i]) * base_w[o,i]

B=65536, in=out=128, G=8, centers = linspace(-1,1,8), w = 2/7.

Strategy (data-parallel over batch, weights replicated):
- Host: transpose x to feature-major [128, B], shard columns 8 ways; fold the
  Gaussian factorization constants e^(7g-g^2) into the coeffs.
- Device, per core (u = (x+1)/w): basis_g = e^(-(u-g)^2) = p * s^g * const
  with p = exp(-u^2) (ScalarE Square+Exp), s = exp(7x) (ScalarE Exp).
  VectorE builds the power chain t_g = t_{g-1} * s (bf16 2x-mode);
  TensorE contracts 8 Gaussian K-tiles g-major per chunk, plus one silu
  K-tile per 512-col psum tile.
- PSUM plan (8 banks, 16 x 512-col f32 tiles):
  tiles 0-7  close at g=7 (chain-paced, early), drain via ScalarE copies
             that ride in the exp-phase pacing slack -> "out";
  tiles 8-15 stay open and are closed by their silu matmul (which can only
             exist after the one exp->silu table switch), drained late;
  tiles 0-7's silu contribution runs afterwards as single-MM groups in the
  banks freed by tiles 8-15, drained to a separate partial "outs" that the
  host adds during the unshard.
- All scalar exp-table work (s, q, t0) runs before the single table switch;
  silu acts after.  Late psum drains are split between ScalarE and VectorE
  tails.  The PE clock gate is warmed by memset-fed matmuls.
"""

import numpy as np

BATCH = 65536
GRID = 8
NCORES = 8
BLOC = BATCH // NCORES  # 8192 batch columns per core
FDP = 512  # psum tile (one bank)
NSUB = BLOC // FDP  # 16 psum tiles
G1 = 8  # tiles 0..G1-1 close early at g=7; the rest close via silu MM
W = 2.0 / (GRID - 1)

# elementwise chunks (fixed 2048-wide tiles) split into pieces; narrow first
# pieces for early start, narrow tail pieces for a short drain.
FDE = 2048
CHUNKS = [[512, 512, 1024], [2048], [2048], [1024, 1024]]

_NC = None


def _build():
    import concourse.mybir as mybir
    from concourse import bacc
    from concourse.tile import TileContext, add_dep_helper

    AF = mybir.ActivationFunctionType
    bf16 = mybir.dt.bfloat16
    f32 = mybir.dt.float32

    nc = bacc.Bacc("TRN2", num_devices=NCORES)
    # NOTE: vector.memset, NOT gpsimd.memset — any gpsimd use forces a ~6us
    # Q7 library load into the preamble that gates the whole kernel.
    cst = nc.alloc_sbuf_tensor("const-float32-bias-c", [128, 1], f32)
    nc.vector.memset(cst.ap(), 1.0 / W)
    nc.const_aps.aps[(f32, 1.0 / W)] = cst.ap()
    nc.all_engine_barrier()
    xt = nc.dram_tensor("xt", [128, BLOC], f32, kind="ExternalInput").ap()
    wt = nc.dram_tensor("wt", [128, 9 * 128], bf16, kind="ExternalInput").ap()
    out = nc.dram_tensor("out", [128, BLOC], bf16, kind="ExternalOutput").ap()
    outs = nc.dram_tensor(
        "outs", [128, G1 * FDP], bf16, kind="ExternalOutput"
    ).ap()

    with TileContext(nc) as tc:
        with (
            tc.tile_pool(name="const", bufs=1) as cpool,
            tc.tile_pool(name="work", bufs=2) as wpool,
            tc.tile_pool(name="obuf", bufs=8) as opool,
            tc.tile_pool(name="psum", bufs=8, space="PSUM") as ppool,
        ):
            # Dummy activation with no DMA deps: forces the exp_and_others
            # ACT table load into the preamble.
            warm_act = cpool.tile([128, 1], f32, name="warm_act")
            nc.vector.memset(warm_act[:], 0.0)
            nc.scalar.activation(warm_act[:], warm_act[:], AF.Exp, scale=1.0)

            # PE HAM clock warm: memset-fed matmuls (no DMA deps).
            wm_s = cpool.tile([128, 128], bf16, name="wm_s")
            wm_m = cpool.tile([128, FDP], bf16, name="wm_m")
            nc.vector.memset(wm_s[:], 0.25)
            nc.vector.memset(wm_m[:], 0.25)
            warm_ps = ppool.tile([128, FDP], f32, name="warm_ps", tag="psum")
            for _ in range(12):
                nc.tensor.matmul(
                    warm_ps[:], wm_s[:], wm_m[:], start=True, stop=True
                )

            # x stream (and weights after the second piece) on the sync queue.
            w_sb = cpool.tile([128, 9, 128], bf16, name="w_sb")
            x_all = cpool.tile([128, BLOC], f32, name="x_all")
            lo = 0
            for i, wd in enumerate([p for ch in CHUNKS for p in ch]):
                nc.sync.dma_start(x_all[:, lo : lo + wd], xt[:, lo : lo + wd])
                lo += wd
                if i == 1:
                    nc.sync.dma_start(
                        w_sb[:], wt.rearrange("p (g o) -> p g o", g=9)
                    )

            # ---- exp phase (scalar) + chain (vector) + gauss MMs (tensor),
            # piece by piece; gen-1 tiles close at g=7 and their ScalarE
            # drain copies are interleaved into the scalar stream.
            psums = [None] * NSUB
            obufs = [None] * NSUB
            prev_op = {"s": None, "v": None}

            def order(eng, op):
                # pin per-engine execution order to emission order so the
                # scheduler cannot shuffle phases (e.g. silu acts into the
                # exp-table phase, which costs ~2.7us per extra table load).
                if prev_op[eng] is not None:
                    add_dep_helper(op.ins, prev_op[eng].ins, False, "order")
                prev_op[eng] = op
                return op

            def emit_copy(k, engine):
                ob = opool.tile([128, FDP], bf16, tag="ob", name=f"ob_{k}")
                obufs[k] = ob
                if engine == "s":
                    order("s", nc.scalar.copy(ob[:], psums[k][:]))
                else:
                    order("v", nc.vector.tensor_copy(ob[:], psums[k][:]))
                nc.scalar.dma_start(out[:, k * FDP : (k + 1) * FDP], ob[:])

            lo = 0
            for c, pieces in enumerate(CHUNKS):
                s = wpool.tile([128, FDE], bf16, tag="s", name=f"s_{c}")
                q = wpool.tile([128, FDE], f32, tag="q", name=f"q_{c}")
                tg = [
                    wpool.tile([128, FDE], bf16, tag=f"t{g}", name=f"t{g}_{c}")
                    for g in range(GRID)
                ]
                off = 0
                for wd in pieces:
                    hs = slice(off, off + wd)
                    xc = x_all[:, lo + off : lo + off + wd]
                    order(
                        "s", nc.scalar.activation(s[:, hs], xc, AF.Exp, scale=2.0 / W)
                    )
                    order(
                        "s",
                        nc.scalar.activation(
                            q[:, hs], xc, AF.Square, bias=1.0 / W, scale=1.0 / W
                        ),
                    )
                    order(
                        "s",
                        nc.scalar.activation(tg[0][:, hs], q[:, hs], AF.Exp, scale=-1.0),
                    )
                    for g in range(1, GRID):
                        order(
                            "v",
                            nc.vector.tensor_mul(
                                tg[g][:, hs], tg[g - 1][:, hs], s[:, hs]
                            ),
                        )
                    # tensor: g-major over this piece's psum tiles
                    ntile = wd // FDP
                    k0 = (lo + off) // FDP
                    for k in range(k0, k0 + ntile):
                        psums[k] = ppool.tile(
                            [128, FDP], f32, tag="psum", name=f"psum_{k}"
                        )
                    for g in range(GRID):
                        for k in range(k0, k0 + ntile):
                            mlo = off + (k - k0) * FDP
                            nc.tensor.matmul(
                                psums[k][:],
                                w_sb[:, g, :],
                                tg[g][:, mlo : mlo + FDP],
                                start=(g == 0),
                                stop=(g == GRID - 1 and k < G1),
                            )
                    # early drains for gen-1 tiles (ride in scalar slack)
                    for k in range(k0, min(k0 + ntile, G1)):
                        emit_copy(k, "s")
                    off += wd
                lo += FDE

            # ---- silu phase: one table switch, acts for gen-2 cols first.
            # 1024-wide acts (two psum tiles each) amortize the ~220ns/act
            # fixed cost; the per-engine order chain keeps them after every
            # exp-table op.
            silu_sb = cpool.tile([128, BLOC], bf16, name="silu_sb")
            FDS = 1024
            act_order = [k for k in range(G1, NSUB, 2)] + [
                k for k in range(0, G1, 2)
            ]
            for k in act_order:
                ks = slice(k * FDP, k * FDP + FDS)
                order(
                    "s",
                    nc.scalar.activation(silu_sb[:, ks], x_all[:, ks], AF.Silu),
                )

            # gen-2 silu MMs close their groups; drains split scalar/vector.
            for k in range(G1, NSUB):
                ks = slice(k * FDP, (k + 1) * FDP)
                nc.tensor.matmul(
                    psums[k][:],
                    w_sb[:, 8, :],
                    silu_sb[:, ks],
                    start=False,
                    stop=True,
                )
            for j, k in enumerate(range(G1, NSUB)):
                emit_copy(k, "v" if j % 2 == 0 else "s")

            # gen-1 silu partial: single-MM groups in banks freed by the
            # gen-2 drains, drained to the separate partial output.
            for i in range(G1):
                ps = ppool.tile([128, FDP], f32, tag="psum", name=f"psilu_{i}")
                ks = slice(i * FDP, (i + 1) * FDP)
                nc.tensor.matmul(
                    ps[:], w_sb[:, 8, :], silu_sb[:, ks], start=True, stop=True
                )
                ob = opool.tile([128, FDP], bf16, tag="ob", name=f"obs_{i}")
                if i % 2 == 0:
                    order("v", nc.vector.tensor_copy(ob[:], ps[:]))
                else:
                    order("s", nc.scalar.copy(ob[:], ps[:]))
                nc.scalar.dma_start(outs[:, ks], ob[:])

    nc.compile()
    return nc


def _prep_weights(coeffs, base_w):
    import ml_dtypes

    g = np.arange(GRID, dtype=np.float64)
    K = np.exp(7.0 * g - g * g)  # t_g = basis_g * e^(g^2-7g) -> fold inverse
    blocks = [
        (coeffs[:, :, gi].astype(np.float64) * K[gi]).T for gi in range(GRID)
    ]  # [in, out] each
    blocks.append(base_w.astype(np.float64).T)
    wtm = np.concatenate(blocks, axis=1)  # [128, 9*128]
    return np.ascontiguousarray(wtm.astype(ml_dtypes.bfloat16))


def _gather(results):
    """Merge per-core outputs: out + silu partial for the first G1 tiles."""
    cols = []
    for c in range(NCORES):
        full = results[c]["out"].astype(np.float32)  # [128, BLOC]
        part = results[c]["outs"].astype(np.float32)  # [128, G1*FDP]
        full[:, : G1 * FDP] += part
        cols.append(full)
    return np.ascontiguousarray(np.concatenate(cols, axis=1).T)


def kernel(x, coeffs, base_w, centers):
    from concourse.bass_utils import run_bass_kernel_spmd

    global _NC
    if _NC is None:
        _NC = _build()

    wtm = _prep_weights(coeffs, base_w)
    xT = np.ascontiguousarray(np.asarray(x, dtype=np.float32).T)  # [128, B]
    in_maps = [
        {
            "xt": np.ascontiguousarray(xT[:, c * BLOC : (c + 1) * BLOC]),
            "wt": wtm,
        }
        for c in range(NCORES)
    ]
    res = run_bass_kernel_spmd(_NC, in_maps, list(range(NCORES)))
    return _gather(res.results)


# revision 14
# speedup vs baseline: 1.1218x; 1.1218x over previous
"""AdaptiveGridKANLayer on 8 TRN2 NeuronCores.

out[b,o] = sum_i sum_g exp(-((x[b,i]-c_g)/w)^2) * coeffs[o,i,g]
         + sum_i silu(x[b,i]) * base_w[o,i]

B=65536, in=out=128, G=8, centers = linspace(-1,1,8), w = 2/7.

Strategy (data-parallel over batch, weights replicated):
- Host: transpose x to feature-major [128, B], shard columns 8 ways; fold the
  Gaussian factorization constants e^(7g-g^2) into the coeffs.
- Device, per core (u = (x+1)/w): basis_g = e^(-(u-g)^2) = p * s^g * const
  with p = exp(-u^2) (ScalarE Square+Exp), s = exp(7x) (ScalarE Exp).
  VectorE builds the power chain t_g = t_{g-1} * s (bf16 2x-mode);
  TensorE contracts 8 Gaussian K-tiles g-major per chunk, plus one silu
  K-tile per 512-col psum tile.
- PSUM plan (8 banks, 16 x 512-col f32 tiles):
  tiles 0-7  close at g=7 (chain-paced, early), drain via ScalarE copies
             that ride in the exp-phase pacing slack -> "out";
  tiles 8-15 stay open and are closed by their silu matmul (which can only
             exist after the one exp->silu table switch), drained late;
  tiles 0-7's silu contribution runs afterwards as single-MM groups in the
  banks freed by tiles 8-15, drained to a separate partial "outs" that the
  host adds during the unshard.
- All scalar exp-table work (s, q, t0) runs before the single table switch;
  silu acts after.  Late psum drains are split between ScalarE and VectorE
  tails.  The PE clock gate is warmed by memset-fed matmuls.
"""

import numpy as np

BATCH = 65536
GRID = 8
NCORES = 8
BLOC = BATCH // NCORES  # 8192 batch columns per core
FDP = 512  # psum tile (one bank)
NSUB = BLOC // FDP  # 16 psum tiles
G1 = 8  # tiles 0..G1-1 close early at g=7; the rest close via silu MM
W = 2.0 / (GRID - 1)

# elementwise chunks (fixed 2048-wide tiles) split into pieces; narrow first
# pieces for early start, narrow tail pieces for a short drain.
FDE = 2048
CHUNKS = [[512, 512, 1024], [2048], [2048], [2048]]

_NC = None


def _build():
    import concourse.mybir as mybir
    from concourse import bacc
    from concourse.tile import TileContext, add_dep_helper

    AF = mybir.ActivationFunctionType
    bf16 = mybir.dt.bfloat16
    f32 = mybir.dt.float32

    nc = bacc.Bacc("TRN2", num_devices=NCORES)
    # NOTE: vector.memset, NOT gpsimd.memset — any gpsimd use forces a ~6us
    # Q7 library load into the preamble that gates the whole kernel.
    cst = nc.alloc_sbuf_tensor("const-float32-bias-c", [128, 1], f32)
    nc.vector.memset(cst.ap(), 1.0 / W)
    nc.const_aps.aps[(f32, 1.0 / W)] = cst.ap()
    nc.all_engine_barrier()
    xt = nc.dram_tensor("xt", [128, BLOC], f32, kind="ExternalInput").ap()
    wt = nc.dram_tensor("wt", [128, 9 * 128], bf16, kind="ExternalInput").ap()
    out = nc.dram_tensor("out", [128, BLOC], bf16, kind="ExternalOutput").ap()
    outs = nc.dram_tensor(
        "outs", [128, G1 * FDP], bf16, kind="ExternalOutput"
    ).ap()

    with TileContext(nc) as tc:
        with (
            tc.tile_pool(name="const", bufs=1) as cpool,
            tc.tile_pool(name="work", bufs=2) as wpool,
            tc.tile_pool(name="obuf", bufs=8) as opool,
            tc.tile_pool(name="psum", bufs=8, space="PSUM") as ppool,
        ):
            # Dummy activation with no DMA deps: forces the exp_and_others
            # ACT table load into the preamble.
            warm_act = cpool.tile([128, 1], f32, name="warm_act")
            nc.vector.memset(warm_act[:], 0.0)
            nc.scalar.activation(warm_act[:], warm_act[:], AF.Exp, scale=1.0)

            # PE HAM clock warm: memset-fed matmuls (no DMA deps).
            wm_s = cpool.tile([128, 128], bf16, name="wm_s")
            wm_m = cpool.tile([128, FDP], bf16, name="wm_m")
            nc.vector.memset(wm_s[:], 0.25)
            nc.vector.memset(wm_m[:], 0.25)
            warm_ps = ppool.tile([128, FDP], f32, name="warm_ps", tag="psum")
            for _ in range(12):
                nc.tensor.matmul(
                    warm_ps[:], wm_s[:], wm_m[:], start=True, stop=True
                )

            # x stream (and weights after the second piece) on the sync queue.
            w_sb = cpool.tile([128, 9, 128], bf16, name="w_sb")
            x_all = cpool.tile([128, BLOC], f32, name="x_all")
            lo = 0
            for i, wd in enumerate([p for ch in CHUNKS for p in ch]):
                nc.sync.dma_start(x_all[:, lo : lo + wd], xt[:, lo : lo + wd])
                lo += wd
                if i == 1:
                    nc.sync.dma_start(
                        w_sb[:], wt.rearrange("p (g o) -> p g o", g=9)
                    )

            # ---- exp phase (scalar) + chain (vector) + gauss MMs (tensor),
            # piece by piece; gen-1 tiles close at g=7 and their ScalarE
            # drain copies are interleaved into the scalar stream.
            psums = [None] * NSUB
            obufs = [None] * NSUB
            prev_op = {"s": None, "v": None}

            def order(eng, op):
                # pin per-engine execution order to emission order so the
                # scheduler cannot shuffle phases (e.g. silu acts into the
                # exp-table phase, which costs ~2.7us per extra table load).
                if prev_op[eng] is not None:
                    add_dep_helper(op.ins, prev_op[eng].ins, False, "order")
                prev_op[eng] = op
                return op

            def emit_copy(k, engine):
                # NOT in the order chain: a copy waits on tensor results, and
                # pinning it into the act stream stalls the whole engine
                # queue.  The scheduler slots it into pacing slack instead.
                ob = opool.tile([128, FDP], bf16, tag="ob", name=f"ob_{k}")
                obufs[k] = ob
                if engine == "s":
                    nc.scalar.copy(ob[:], psums[k][:])
                else:
                    nc.vector.tensor_copy(ob[:], psums[k][:])
                nc.scalar.dma_start(out[:, k * FDP : (k + 1) * FDP], ob[:])

            lo = 0
            for c, pieces in enumerate(CHUNKS):
                s = wpool.tile([128, FDE], bf16, tag="s", name=f"s_{c}")
                q = wpool.tile([128, FDE], f32, tag="q", name=f"q_{c}")
                tg = [
                    wpool.tile([128, FDE], bf16, tag=f"t{g}", name=f"t{g}_{c}")
                    for g in range(GRID)
                ]
                off = 0
                for wd in pieces:
                    hs = slice(off, off + wd)
                    xc = x_all[:, lo + off : lo + off + wd]
                    order(
                        "s", nc.scalar.activation(s[:, hs], xc, AF.Exp, scale=2.0 / W)
                    )
                    order(
                        "s",
                        nc.scalar.activation(
                            q[:, hs], xc, AF.Square, bias=1.0 / W, scale=1.0 / W
                        ),
                    )
                    order(
                        "s",
                        nc.scalar.activation(tg[0][:, hs], q[:, hs], AF.Exp, scale=-1.0),
                    )
                    for g in range(1, GRID):
                        order(
                            "v",
                            nc.vector.tensor_mul(
                                tg[g][:, hs], tg[g - 1][:, hs], s[:, hs]
                            ),
                        )
                    # tensor: g-major over this piece's psum tiles
                    ntile = wd // FDP
                    k0 = (lo + off) // FDP
                    for k in range(k0, k0 + ntile):
                        psums[k] = ppool.tile(
                            [128, FDP], f32, tag="psum", name=f"psum_{k}"
                        )
                    for g in range(GRID):
                        for k in range(k0, k0 + ntile):
                            mlo = off + (k - k0) * FDP
                            nc.tensor.matmul(
                                psums[k][:],
                                w_sb[:, g, :],
                                tg[g][:, mlo : mlo + FDP],
                                start=(g == 0),
                                stop=(g == GRID - 1 and k < G1),
                            )
                    # early drains for gen-1 tiles (ride in scalar slack)
                    for k in range(k0, min(k0 + ntile, G1)):
                        emit_copy(k, "s")
                    off += wd
                lo += FDE

            # ---- silu phase: one table switch, acts for gen-2 cols first.
            # 1024-wide acts (two psum tiles each) amortize the ~220ns/act
            # fixed cost; the per-engine order chain keeps them after every
            # exp-table op.
            silu_sb = cpool.tile([128, BLOC], bf16, name="silu_sb")
            FDS = 2048
            act_order = [k for k in range(G1, NSUB, 4)] + [
                k for k in range(0, G1, 4)
            ]
            for k in act_order:
                ks = slice(k * FDP, k * FDP + FDS)
                order(
                    "s",
                    nc.scalar.activation(silu_sb[:, ks], x_all[:, ks], AF.Silu),
                )

            # gen-2 silu MMs close their groups; drains split scalar/vector.
            for k in range(G1, NSUB):
                ks = slice(k * FDP, (k + 1) * FDP)
                nc.tensor.matmul(
                    psums[k][:],
                    w_sb[:, 8, :],
                    silu_sb[:, ks],
                    start=False,
                    stop=True,
                )
            for j, k in enumerate(range(G1, NSUB)):
                emit_copy(k, "v" if j % 2 == 0 else "s")

            # gen-1 silu partial: single-MM groups in banks freed by the
            # gen-2 drains, drained to the separate partial output.
            for i in range(G1):
                ps = ppool.tile([128, FDP], f32, tag="psum", name=f"psilu_{i}")
                ks = slice(i * FDP, (i + 1) * FDP)
                nc.tensor.matmul(
                    ps[:], w_sb[:, 8, :], silu_sb[:, ks], start=True, stop=True
                )
                ob = opool.tile([128, FDP], bf16, tag="ob", name=f"obs_{i}")
                if i % 2 == 0:
                    nc.vector.tensor_copy(ob[:], ps[:])
                else:
                    nc.scalar.copy(ob[:], ps[:])
                nc.scalar.dma_start(outs[:, ks], ob[:])

    nc.compile()
    return nc


def _prep_weights(coeffs, base_w):
    import ml_dtypes

    g = np.arange(GRID, dtype=np.float64)
    K = np.exp(7.0 * g - g * g)  # t_g = basis_g * e^(g^2-7g) -> fold inverse
    blocks = [
        (coeffs[:, :, gi].astype(np.float64) * K[gi]).T for gi in range(GRID)
    ]  # [in, out] each
    blocks.append(base_w.astype(np.float64).T)
    wtm = np.concatenate(blocks, axis=1)  # [128, 9*128]
    return np.ascontiguousarray(wtm.astype(ml_dtypes.bfloat16))


def _gather(results):
    """Merge per-core outputs: out + silu partial for the first G1 tiles."""
    cols = []
    for c in range(NCORES):
        full = results[c]["out"].astype(np.float32)  # [128, BLOC]
        part = results[c]["outs"].astype(np.float32)  # [128, G1*FDP]
        full[:, : G1 * FDP] += part
        cols.append(full)
    return np.ascontiguousarray(np.concatenate(cols, axis=1).T)


def kernel(x, coeffs, base_w, centers):
    from concourse.bass_utils import run_bass_kernel_spmd

    global _NC
    if _NC is None:
        _NC = _build()

    wtm = _prep_weights(coeffs, base_w)
    xT = np.ascontiguousarray(np.asarray(x, dtype=np.float32).T)  # [128, B]
    in_maps = [
        {
            "xt": np.ascontiguousarray(xT[:, c * BLOC : (c + 1) * BLOC]),
            "wt": wtm,
        }
        for c in range(NCORES)
    ]
    res = run_bass_kernel_spmd(_NC, in_maps, list(range(NCORES)))
    return _gather(res.results)


# revision 15
# speedup vs baseline: 1.3357x; 1.1907x over previous
"""AdaptiveGridKANLayer on 8 TRN2 NeuronCores.

out[b,o] = sum_i sum_g exp(-((x[b,i]-c_g)/w)^2) * coeffs[o,i,g]
         + sum_i silu(x[b,i]) * base_w[o,i]

B=65536, in=out=128, G=8, centers = linspace(-1,1,8), w = 2/7.

Strategy (data-parallel over batch, weights replicated):
- Host: transpose x to feature-major [128, B], shard columns 8 ways; fold the
  Gaussian factorization constants e^(7g-g^2) into the coeffs.
- Device, per core (u = (x+1)/w): basis_g = e^(-(u-g)^2) = p * s^g * const
  with p = exp(-u^2) (ScalarE Square+Exp), s = exp(7x) (ScalarE Exp).
  VectorE builds the power chain t_g = t_{g-1} * s (bf16 2x-mode);
  TensorE contracts 8 Gaussian K-tiles g-major per chunk, plus one silu
  K-tile per 512-col psum tile.
- PSUM plan (8 banks, 16 x 512-col f32 tiles):
  tiles 0-7  close at g=7 (chain-paced, early), drain via ScalarE copies
             that ride in the exp-phase pacing slack -> "out";
  tiles 8-15 stay open and are closed by their silu matmul (which can only
             exist after the one exp->silu table switch), drained late;
  tiles 0-7's silu contribution runs afterwards as single-MM groups in the
  banks freed by tiles 8-15, drained to a separate partial "outs" that the
  host adds during the unshard.
- All scalar exp-table work (s, q, t0) runs before the single table switch;
  silu acts after.  Late psum drains are split between ScalarE and VectorE
  tails.  The PE clock gate is warmed by memset-fed matmuls.
"""

import numpy as np

BATCH = 65536
GRID = 8
NCORES = 8
BLOC = BATCH // NCORES  # 8192 batch columns per core
FDP = 512  # psum tile (one bank)
NSUB = BLOC // FDP  # 16 psum tiles
G1 = 8  # tiles 0..G1-1 close early at g=7; the rest close via silu MM
W = 2.0 / (GRID - 1)

# elementwise chunks (fixed 2048-wide tiles) split into pieces; narrow first
# pieces for early start, narrow tail pieces for a short drain.
FDE = 2048
CHUNKS = [[512, 512, 1024], [2048], [2048], [2048]]

_NC = None


def _build():
    import concourse.mybir as mybir
    from concourse import bacc
    from concourse.tile import TileContext, add_dep_helper

    AF = mybir.ActivationFunctionType
    bf16 = mybir.dt.bfloat16
    f32 = mybir.dt.float32

    nc = bacc.Bacc("TRN2", num_devices=NCORES)
    # NOTE: vector.memset, NOT gpsimd.memset — any gpsimd use forces a ~6us
    # Q7 library load into the preamble that gates the whole kernel.
    cst = nc.alloc_sbuf_tensor("const-float32-bias-c", [128, 1], f32)
    nc.vector.memset(cst.ap(), 1.0 / W)
    nc.const_aps.aps[(f32, 1.0 / W)] = cst.ap()
    nc.all_engine_barrier()
    xt = nc.dram_tensor("xt", [128, BLOC], f32, kind="ExternalInput").ap()
    wt = nc.dram_tensor("wt", [128, 9 * 128], bf16, kind="ExternalInput").ap()
    out = nc.dram_tensor("out", [128, BLOC], bf16, kind="ExternalOutput").ap()
    outs = nc.dram_tensor(
        "outs", [128, G1 * FDP], bf16, kind="ExternalOutput"
    ).ap()

    with TileContext(nc) as tc:
        with (
            tc.tile_pool(name="const", bufs=1) as cpool,
            tc.tile_pool(name="work", bufs=2) as wpool,
            tc.tile_pool(name="obuf", bufs=8) as opool,
            tc.tile_pool(name="psum", bufs=8, space="PSUM") as ppool,
        ):
            # Dummy activation with no DMA deps: forces the exp_and_others
            # ACT table load into the preamble.
            warm_act = cpool.tile([128, 1], f32, name="warm_act")
            nc.vector.memset(warm_act[:], 0.0)
            nc.scalar.activation(warm_act[:], warm_act[:], AF.Exp, scale=1.0)

            # PE HAM clock warm: memset-fed matmuls (no DMA deps).
            wm_s = cpool.tile([128, 128], bf16, name="wm_s")
            wm_m = cpool.tile([128, FDP], bf16, name="wm_m")
            nc.vector.memset(wm_s[:], 0.25)
            nc.vector.memset(wm_m[:], 0.25)
            warm_ps = ppool.tile([128, FDP], f32, name="warm_ps", tag="psum")
            for _ in range(12):
                nc.tensor.matmul(
                    warm_ps[:], wm_s[:], wm_m[:], start=True, stop=True
                )

            # x stream (and weights after the second piece) on the sync queue.
            w_sb = cpool.tile([128, 9, 128], bf16, name="w_sb")
            x_all = cpool.tile([128, BLOC], f32, name="x_all")
            lo = 0
            for i, wd in enumerate([p for ch in CHUNKS for p in ch]):
                nc.sync.dma_start(x_all[:, lo : lo + wd], xt[:, lo : lo + wd])
                lo += wd
                if i == 1:
                    nc.sync.dma_start(
                        w_sb[:], wt.rearrange("p (g o) -> p g o", g=9)
                    )

            # ---- exp phase (scalar) + chain (vector) + gauss MMs (tensor),
            # piece by piece; gen-1 tiles close at g=7 and their ScalarE
            # drain copies are interleaved into the scalar stream.
            psums = [None] * NSUB
            obufs = [None] * NSUB
            prev_op = {"s": None, "v": None}

            def order(eng, op):
                # pin per-engine execution order to emission order so the
                # scheduler cannot shuffle phases (e.g. silu acts into the
                # exp-table phase, which costs ~2.7us per extra table load).
                if prev_op[eng] is not None:
                    add_dep_helper(op.ins, prev_op[eng].ins, False, "order")
                prev_op[eng] = op
                return op

            def emit_copy(k, engine):
                # NOT in the order chain: a copy waits on tensor results, and
                # pinning it into the act stream stalls the whole engine
                # queue.  The scheduler slots it into pacing slack instead.
                ob = opool.tile([128, FDP], bf16, tag="ob", name=f"ob_{k}")
                obufs[k] = ob
                if engine == "s":
                    nc.scalar.copy(ob[:], psums[k][:])
                else:
                    nc.vector.tensor_copy(ob[:], psums[k][:])
                nc.scalar.dma_start(out[:, k * FDP : (k + 1) * FDP], ob[:])

            lo = 0
            for c, pieces in enumerate(CHUNKS):
                s = wpool.tile([128, FDE], bf16, tag="s", name=f"s_{c}")
                q = wpool.tile([128, FDE], f32, tag="q", name=f"q_{c}")
                tg = [
                    wpool.tile([128, FDE], bf16, tag=f"t{g}", name=f"t{g}_{c}")
                    for g in range(GRID)
                ]
                off = 0
                for wd in pieces:
                    hs = slice(off, off + wd)
                    xc = x_all[:, lo + off : lo + off + wd]
                    order(
                        "s", nc.scalar.activation(s[:, hs], xc, AF.Exp, scale=2.0 / W)
                    )
                    order(
                        "s",
                        nc.scalar.activation(
                            q[:, hs], xc, AF.Square, bias=1.0 / W, scale=1.0 / W
                        ),
                    )
                    order(
                        "s",
                        nc.scalar.activation(tg[0][:, hs], q[:, hs], AF.Exp, scale=-1.0),
                    )
                    for g in range(1, GRID):
                        order(
                            "v",
                            nc.vector.tensor_mul(
                                tg[g][:, hs], tg[g - 1][:, hs], s[:, hs]
                            ),
                        )
                    # tensor: tile-major over this piece's psum tiles
                    ntile = wd // FDP
                    k0 = (lo + off) // FDP
                    for k in range(k0, k0 + ntile):
                        psums[k] = ppool.tile(
                            [128, FDP], f32, tag="psum", name=f"psum_{k}"
                        )
                    for k in range(k0, k0 + ntile):
                        for g in range(GRID):
                            mlo = off + (k - k0) * FDP
                            nc.tensor.matmul(
                                psums[k][:],
                                w_sb[:, g, :],
                                tg[g][:, mlo : mlo + FDP],
                                start=(g == 0),
                                stop=(g == GRID - 1 and k < G1),
                            )
                    # early drains for gen-1 tiles (ride in scalar slack)
                    for k in range(k0, min(k0 + ntile, G1)):
                        emit_copy(k, "s")
                    off += wd
                lo += FDE

            # ---- silu phase: one table switch, acts for gen-2 cols first.
            # 1024-wide acts (two psum tiles each) amortize the ~220ns/act
            # fixed cost; the per-engine order chain keeps them after every
            # exp-table op.
            silu_sb = cpool.tile([128, BLOC], bf16, name="silu_sb")
            FDS = 2048
            act_order = [k for k in range(G1, NSUB, 4)] + [
                k for k in range(0, G1, 4)
            ]
            for k in act_order:
                ks = slice(k * FDP, k * FDP + FDS)
                order(
                    "s",
                    nc.scalar.activation(silu_sb[:, ks], x_all[:, ks], AF.Silu),
                )

            # gen-2 silu MMs close their groups; drains split scalar/vector.
            for k in range(G1, NSUB):
                ks = slice(k * FDP, (k + 1) * FDP)
                nc.tensor.matmul(
                    psums[k][:],
                    w_sb[:, 8, :],
                    silu_sb[:, ks],
                    start=False,
                    stop=True,
                )
            for j, k in enumerate(range(G1, NSUB)):
                emit_copy(k, "v" if j % 2 == 0 else "s")

            # gen-1 silu partial: single-MM groups in banks freed by the
            # gen-2 drains, drained to the separate partial output.
            for i in range(G1):
                ps = ppool.tile([128, FDP], f32, tag="psum", name=f"psilu_{i}")
                ks = slice(i * FDP, (i + 1) * FDP)
                nc.tensor.matmul(
                    ps[:], w_sb[:, 8, :], silu_sb[:, ks], start=True, stop=True
                )
                ob = opool.tile([128, FDP], bf16, tag="ob", name=f"obs_{i}")
                if i % 2 == 0:
                    nc.vector.tensor_copy(ob[:], ps[:])
                else:
                    nc.scalar.copy(ob[:], ps[:])
                nc.scalar.dma_start(outs[:, ks], ob[:])

    nc.compile()
    return nc


def _prep_weights(coeffs, base_w):
    import ml_dtypes

    g = np.arange(GRID, dtype=np.float64)
    K = np.exp(7.0 * g - g * g)  # t_g = basis_g * e^(g^2-7g) -> fold inverse
    blocks = [
        (coeffs[:, :, gi].astype(np.float64) * K[gi]).T for gi in range(GRID)
    ]  # [in, out] each
    blocks.append(base_w.astype(np.float64).T)
    wtm = np.concatenate(blocks, axis=1)  # [128, 9*128]
    return np.ascontiguousarray(wtm.astype(ml_dtypes.bfloat16))


def _gather(results):
    """Merge per-core outputs: out + silu partial for the first G1 tiles."""
    cols = []
    for c in range(NCORES):
        full = results[c]["out"].astype(np.float32)  # [128, BLOC]
        part = results[c]["outs"].astype(np.float32)  # [128, G1*FDP]
        full[:, : G1 * FDP] += part
        cols.append(full)
    return np.ascontiguousarray(np.concatenate(cols, axis=1).T)


def kernel(x, coeffs, base_w, centers):
    from concourse.bass_utils import run_bass_kernel_spmd

    global _NC
    if _NC is None:
        _NC = _build()

    wtm = _prep_weights(coeffs, base_w)
    xT = np.ascontiguousarray(np.asarray(x, dtype=np.float32).T)  # [128, B]
    in_maps = [
        {
            "xt": np.ascontiguousarray(xT[:, c * BLOC : (c + 1) * BLOC]),
            "wt": wtm,
        }
        for c in range(NCORES)
    ]
    res = run_bass_kernel_spmd(_NC, in_maps, list(range(NCORES)))
    return _gather(res.results)


# revision 16
# speedup vs baseline: 1.3638x; 1.0211x over previous
"""AdaptiveGridKANLayer on 8 TRN2 NeuronCores.

out[b,o] = sum_i sum_g exp(-((x[b,i]-c_g)/w)^2) * coeffs[o,i,g]
         + sum_i silu(x[b,i]) * base_w[o,i]

B=65536, in=out=128, G=8, centers = linspace(-1,1,8), w = 2/7.

Strategy (data-parallel over batch, weights replicated):
- Host: transpose x to feature-major [128, B], shard columns 8 ways; fold the
  Gaussian factorization constants e^(7g-g^2) into the coeffs.
- Device, per core (u = (x+1)/w): basis_g = e^(-(u-g)^2) = p * s^g * const
  with p = exp(-u^2) (ScalarE Square+Exp), s = exp(7x) (ScalarE Exp).
  VectorE builds the power chain t_g = t_{g-1} * s (bf16 2x-mode);
  TensorE contracts 8 Gaussian K-tiles g-major per chunk, plus one silu
  K-tile per 512-col psum tile.
- PSUM plan (8 banks, 16 x 512-col f32 tiles):
  tiles 0-7  close at g=7 (chain-paced, early), drain via ScalarE copies
             that ride in the exp-phase pacing slack -> "out";
  tiles 8-15 stay open and are closed by their silu matmul (which can only
             exist after the one exp->silu table switch), drained late;
  tiles 0-7's silu contribution runs afterwards as single-MM groups in the
  banks freed by tiles 8-15, drained to a separate partial "outs" that the
  host adds during the unshard.
- All scalar exp-table work (s, q, t0) runs before the single table switch;
  silu acts after.  Late psum drains are split between ScalarE and VectorE
  tails.  The PE clock gate is warmed by memset-fed matmuls.
"""

import numpy as np

BATCH = 65536
GRID = 8
NCORES = 8
BLOC = BATCH // NCORES  # 8192 batch columns per core
FDP = 512  # psum tile (one bank)
NSUB = BLOC // FDP  # 16 psum tiles
G1 = 8  # tiles 0..G1-1 close early at g=7; the rest close via silu MM
W = 2.0 / (GRID - 1)

# elementwise chunks (fixed 2048-wide tiles) split into pieces; narrow first
# pieces for early start, narrow tail pieces for a short drain.
FDE = 2048
CHUNKS = [[512, 512, 1024], [2048], [2048], [2048]]

_NC = None


def _build():
    import concourse.mybir as mybir
    from concourse import bacc
    from concourse.tile import TileContext, add_dep_helper

    AF = mybir.ActivationFunctionType
    bf16 = mybir.dt.bfloat16
    f32 = mybir.dt.float32

    nc = bacc.Bacc("TRN2", num_devices=NCORES)
    # NOTE: vector.memset, NOT gpsimd.memset — any gpsimd use forces a ~6us
    # Q7 library load into the preamble that gates the whole kernel.
    cst = nc.alloc_sbuf_tensor("const-float32-bias-c", [128, 1], f32)
    nc.vector.memset(cst.ap(), 1.0 / W)
    nc.const_aps.aps[(f32, 1.0 / W)] = cst.ap()
    nc.all_engine_barrier()
    xt = nc.dram_tensor("xt", [128, BLOC], f32, kind="ExternalInput").ap()
    wt = nc.dram_tensor("wt", [128, 9 * 128], bf16, kind="ExternalInput").ap()
    out = nc.dram_tensor("out", [128, BLOC], bf16, kind="ExternalOutput").ap()
    outs = nc.dram_tensor(
        "outs", [128, G1 * FDP], bf16, kind="ExternalOutput"
    ).ap()

    with TileContext(nc) as tc:
        with (
            tc.tile_pool(name="const", bufs=1) as cpool,
            tc.tile_pool(name="work", bufs=2) as wpool,
            tc.tile_pool(name="obuf", bufs=8) as opool,
            tc.tile_pool(name="psum", bufs=8, space="PSUM") as ppool,
        ):
            # Dummy activation with no DMA deps: forces the exp_and_others
            # ACT table load into the preamble.
            warm_act = cpool.tile([128, 1], f32, name="warm_act")
            nc.vector.memset(warm_act[:], 0.0)
            nc.scalar.activation(warm_act[:], warm_act[:], AF.Exp, scale=1.0)

            # PE HAM clock warm: memset-fed matmuls (no DMA deps).
            wm_s = cpool.tile([128, 128], bf16, name="wm_s")
            wm_m = cpool.tile([128, FDP], bf16, name="wm_m")
            nc.vector.memset(wm_s[:], 0.25)
            nc.vector.memset(wm_m[:], 0.25)
            warm_ps = ppool.tile([128, FDP], f32, name="warm_ps", tag="psum")
            for _ in range(12):
                nc.tensor.matmul(
                    warm_ps[:], wm_s[:], wm_m[:], start=True, stop=True
                )

            # x stream (and weights after the second piece) on the sync queue.
            w_sb = cpool.tile([128, 9, 128], bf16, name="w_sb")
            x_all = cpool.tile([128, BLOC], f32, name="x_all")
            lo = 0
            for i, wd in enumerate([p for ch in CHUNKS for p in ch]):
                nc.sync.dma_start(x_all[:, lo : lo + wd], xt[:, lo : lo + wd])
                lo += wd
                if i == 1:
                    nc.sync.dma_start(
                        w_sb[:], wt.rearrange("p (g o) -> p g o", g=9)
                    )

            # ---- exp phase (scalar) + chain (vector) + gauss MMs (tensor),
            # piece by piece; gen-1 tiles close at g=7 and their ScalarE
            # drain copies are interleaved into the scalar stream.
            psums = [None] * NSUB
            obufs = [None] * NSUB
            prev_op = {"s": None, "v": None}

            def order(eng, op):
                # pin per-engine execution order to emission order so the
                # scheduler cannot shuffle phases (e.g. silu acts into the
                # exp-table phase, which costs ~2.7us per extra table load).
                if prev_op[eng] is not None:
                    add_dep_helper(op.ins, prev_op[eng].ins, False, "order")
                prev_op[eng] = op
                return op

            def emit_copy(k, engine):
                # NOT in the order chain: a copy waits on tensor results, and
                # pinning it into the act stream stalls the whole engine
                # queue.  The scheduler slots it into pacing slack instead.
                ob = opool.tile([128, FDP], bf16, tag="ob", name=f"ob_{k}")
                obufs[k] = ob
                if engine == "s":
                    nc.scalar.copy(ob[:], psums[k][:])
                else:
                    nc.vector.tensor_copy(ob[:], psums[k][:])
                nc.scalar.dma_start(out[:, k * FDP : (k + 1) * FDP], ob[:])

            lo = 0
            for c, pieces in enumerate(CHUNKS):
                s = wpool.tile([128, FDE], bf16, tag="s", bufs=3, name=f"s_{c}")
                q = wpool.tile([128, FDE], f32, tag="q", bufs=3, name=f"q_{c}")
                tg = [
                    wpool.tile(
                        [128, FDE], bf16, tag=f"t{g}", bufs=3, name=f"t{g}_{c}"
                    )
                    for g in range(GRID)
                ]
                off = 0
                for wd in pieces:
                    hs = slice(off, off + wd)
                    xc = x_all[:, lo + off : lo + off + wd]
                    order(
                        "s", nc.scalar.activation(s[:, hs], xc, AF.Exp, scale=2.0 / W)
                    )
                    order(
                        "s",
                        nc.scalar.activation(
                            q[:, hs], xc, AF.Square, bias=1.0 / W, scale=1.0 / W
                        ),
                    )
                    order(
                        "s",
                        nc.scalar.activation(tg[0][:, hs], q[:, hs], AF.Exp, scale=-1.0),
                    )
                    for g in range(1, GRID):
                        order(
                            "v",
                            nc.vector.tensor_mul(
                                tg[g][:, hs], tg[g - 1][:, hs], s[:, hs]
                            ),
                        )
                    # tensor: tile-major over this piece's psum tiles
                    ntile = wd // FDP
                    k0 = (lo + off) // FDP
                    for k in range(k0, k0 + ntile):
                        psums[k] = ppool.tile(
                            [128, FDP], f32, tag="psum", name=f"psum_{k}"
                        )
                    for k in range(k0, k0 + ntile):
                        for g in range(GRID):
                            mlo = off + (k - k0) * FDP
                            nc.tensor.matmul(
                                psums[k][:],
                                w_sb[:, g, :],
                                tg[g][:, mlo : mlo + FDP],
                                start=(g == 0),
                                stop=(g == GRID - 1 and k < G1),
                            )
                    # early drains for gen-1 tiles (ride in scalar slack)
                    for k in range(k0, min(k0 + ntile, G1)):
                        emit_copy(k, "s")
                    off += wd
                lo += FDE

            # ---- silu phase: one table switch, acts for gen-2 cols first.
            # 1024-wide acts (two psum tiles each) amortize the ~220ns/act
            # fixed cost; the per-engine order chain keeps them after every
            # exp-table op.
            silu_sb = cpool.tile([128, BLOC], bf16, name="silu_sb")
            FDS = 2048
            act_order = [k for k in range(G1, NSUB, 4)] + [
                k for k in range(0, G1, 4)
            ]
            for k in act_order:
                ks = slice(k * FDP, k * FDP + FDS)
                order(
                    "s",
                    nc.scalar.activation(silu_sb[:, ks], x_all[:, ks], AF.Silu),
                )

            # gen-2 silu MMs close their groups; drains split scalar/vector.
            for k in range(G1, NSUB):
                ks = slice(k * FDP, (k + 1) * FDP)
                nc.tensor.matmul(
                    psums[k][:],
                    w_sb[:, 8, :],
                    silu_sb[:, ks],
                    start=False,
                    stop=True,
                )
            for j, k in enumerate(range(G1, NSUB)):
                emit_copy(k, "v" if j % 2 == 0 else "s")

            # gen-1 silu partial: single-MM groups in banks freed by the
            # gen-2 drains, drained to the separate partial output.
            for i in range(G1):
                ps = ppool.tile([128, FDP], f32, tag="psum", name=f"psilu_{i}")
                ks = slice(i * FDP, (i + 1) * FDP)
                nc.tensor.matmul(
                    ps[:], w_sb[:, 8, :], silu_sb[:, ks], start=True, stop=True
                )
                ob = opool.tile([128, FDP], bf16, tag="ob", name=f"obs_{i}")
                if i % 2 == 0:
                    nc.vector.tensor_copy(ob[:], ps[:])
                else:
                    nc.scalar.copy(ob[:], ps[:])
                nc.scalar.dma_start(outs[:, ks], ob[:])

    nc.compile()
    return nc


def _prep_weights(coeffs, base_w):
    import ml_dtypes

    g = np.arange(GRID, dtype=np.float64)
    K = np.exp(7.0 * g - g * g)  # t_g = basis_g * e^(g^2-7g) -> fold inverse
    blocks = [
        (coeffs[:, :, gi].astype(np.float64) * K[gi]).T for gi in range(GRID)
    ]  # [in, out] each
    blocks.append(base_w.astype(np.float64).T)
    wtm = np.concatenate(blocks, axis=1)  # [128, 9*128]
    return np.ascontiguousarray(wtm.astype(ml_dtypes.bfloat16))


def _gather(results):
    """Merge per-core outputs: out + silu partial for the first G1 tiles."""
    cols = []
    for c in range(NCORES):
        full = results[c]["out"].astype(np.float32)  # [128, BLOC]
        part = results[c]["outs"].astype(np.float32)  # [128, G1*FDP]
        full[:, : G1 * FDP] += part
        cols.append(full)
    return np.ascontiguousarray(np.concatenate(cols, axis=1).T)


def kernel(x, coeffs, base_w, centers):
    from concourse.bass_utils import run_bass_kernel_spmd

    global _NC
    if _NC is None:
        _NC = _build()

    wtm = _prep_weights(coeffs, base_w)
    xT = np.ascontiguousarray(np.asarray(x, dtype=np.float32).T)  # [128, B]
    in_maps = [
        {
            "xt": np.ascontiguousarray(xT[:, c * BLOC : (c + 1) * BLOC]),
            "wt": wtm,
        }
        for c in range(NCORES)
    ]
    res = run_bass_kernel_spmd(_NC, in_maps, list(range(NCORES)))
    return _gather(res.results)


# revision 23
# speedup vs baseline: 1.4393x; 1.0553x over previous
"""AdaptiveGridKANLayer on 8 TRN2 NeuronCores.

out[b,o] = sum_i sum_g exp(-((x[b,i]-c_g)/w)^2) * coeffs[o,i,g]
         + sum_i silu(x[b,i]) * base_w[o,i]

B=65536, in=out=128, G=8, centers = linspace(-1,1,8), w = 2/7.

Strategy (data-parallel over batch, weights replicated):
- Host: transpose x to feature-major [128, B], shard columns 8 ways; fold the
  Gaussian factorization constants e^(7g-g^2) into the coeffs.
- Device, per core (u = (x+1)/w): basis_g = e^(-(u-g)^2) = p * s^g * const
  with p = exp(-u^2) (ScalarE Square+Exp), s = exp(7x) (ScalarE Exp).
  VectorE builds the power chain t_g = t_{g-1} * s (bf16 2x-mode);
  TensorE contracts tile-major (g inner) per 512-col accumulation group.
- PSUM: 8 banks = 4 rotating slots of 1024-col f32 tiles. Main tiles
  M0..M7; M0..M3 (cols 0..4095) close at g=7 (early, chain-paced) so their
  slots recycle for M4..M7; M4..M7 are closed late by their silu matmuls
  (silu activations exist only after the one exp->silu table switch).
  M0..M3's silu contribution runs afterwards as single-MM groups P0..P3 in
  recycled slots, drained to a separate partial "outs" that the host adds
  during the unshard.
- ScalarE stream order (enforced): exp acts chunk 0..3 with the early M0..M3
  drain copies placed right after later chunks' acts (inside real pacing
  slack, never blocking the chain feed), one table switch, silu acts
  (gen-2 cols first), then its share of late drain copies.  VectorE: chain
  only, then late copies.  All engine op order is pinned (sync=False deps);
  tensor stays tile-major (g-major provokes an SBUF producer-consumer
  conflict that slows DVE/ACT ~20%).
"""

import numpy as np

BATCH = 65536
GRID = 8
NCORES = 8
BLOC = BATCH // NCORES  # 8192 batch columns per core
FDP = 512  # matmul free dim / accumulation group width
TW = 1024  # psum tile width (2 banks); 2 groups per tile
NTILE = BLOC // TW  # 8 main psum tiles
G1 = 4  # main tiles 0..3 close early at g=7; 4..7 close via silu MM
W = 2.0 / (GRID - 1)

FDE = 2048
CHUNKS = [[256, 256, 512, 1024], [2048], [2048], [2048]]

_NC = None


def _build():
    import concourse.mybir as mybir
    from concourse import bacc
    from concourse.tile import TileContext, add_dep_helper

    AF = mybir.ActivationFunctionType
    bf16 = mybir.dt.bfloat16
    f32 = mybir.dt.float32

    nc = bacc.Bacc("TRN2", num_devices=NCORES)
    # Bias constant for the Square activation. Must be a raw (non-pool)
    # tensor: const_aps captures the AP before pool relocation. vector
    # memset, NOT gpsimd (a single gpsimd op drags a ~6us Q7 library load
    # into the preamble).
    cst = nc.alloc_sbuf_tensor("const-float32-bias-c", [128, 1], f32)
    nc.vector.memset(cst.ap(), 1.0 / W)
    nc.const_aps.aps[(f32, 1.0 / W)] = cst.ap()
    nc.all_engine_barrier()
    xt = nc.dram_tensor("xt", [128, BLOC], f32, kind="ExternalInput").ap()
    wt = nc.dram_tensor("wt", [128, 9 * 128], bf16, kind="ExternalInput").ap()
    out = nc.dram_tensor("out", [128, BLOC], bf16, kind="ExternalOutput").ap()
    outs = nc.dram_tensor(
        "outs", [128, G1 * TW], bf16, kind="ExternalOutput"
    ).ap()

    with TileContext(nc) as tc:
        with (
            tc.tile_pool(name="const", bufs=1) as cpool,
            tc.tile_pool(name="work", bufs=3) as wpool,
            tc.tile_pool(name="obuf", bufs=6) as opool,
            tc.tile_pool(name="psum", bufs=4, space="PSUM") as ppool,
        ):
            # Exp table load during preamble (no DMA deps).
            warm_act = cpool.tile([128, 1], f32, name="warm_act")
            nc.vector.memset(warm_act[:], 0.0)
            nc.scalar.activation(warm_act[:], warm_act[:], AF.Exp, scale=1.0)

            # PE HAM clock warm: memset-fed matmuls (no DMA deps).
            wm_s = cpool.tile([128, 128], bf16, name="wm_s")
            wm_m = cpool.tile([128, FDP], bf16, name="wm_m")
            nc.vector.memset(wm_s[:], 0.25)
            nc.vector.memset(wm_m[:], 0.25)
            warm_ps = ppool.tile([128, FDP], f32, name="warm_ps", tag="psum")
            for _ in range(12):
                nc.tensor.matmul(
                    warm_ps[:], wm_s[:], wm_m[:], start=True, stop=True
                )

            # x stream (and weights after the third piece) on the sync queue.
            w_sb = cpool.tile([128, 9, 128], bf16, name="w_sb")
            x_all = cpool.tile([128, BLOC], f32, name="x_all")
            lo = 0
            for i, wd in enumerate([p for ch in CHUNKS for p in ch]):
                nc.sync.dma_start(x_all[:, lo : lo + wd], xt[:, lo : lo + wd])
                lo += wd
                if i == 2:
                    nc.sync.dma_start(
                        w_sb[:], wt.rearrange("p (g o) -> p g o", g=9)
                    )

            psums = [None] * NTILE
            prev_op = {"s": None, "v": None}

            def order(eng, op):
                if prev_op[eng] is not None:
                    add_dep_helper(op.ins, prev_op[eng].ins, False, "order")
                prev_op[eng] = op
                return op

            def emit_copy(m, engine, ordered=True):
                ob = opool.tile([128, TW], bf16, tag="ob", name=f"ob_{m}")
                if engine == "s":
                    op = nc.scalar.copy(ob[:], psums[m][:])
                else:
                    op = nc.vector.tensor_copy(ob[:], psums[m][:])
                if ordered:
                    order(engine, op)
                nc.scalar.dma_start(out[:, m * TW : (m + 1) * TW], ob[:])

            # ---- exp phase: scalar acts, vector chain, tensor tile-major.
            # Gauss matmuls are emitted per completed 512-col block (pieces
            # can be narrower than a block).  Chunk 3's gauss matmuls are
            # deferred so the tensor tail can interleave silu/partial work.
            tgs = [None] * len(CHUNKS)

            def emit_gauss(m, poff, tg, goff, last_stop):
                for g in range(GRID):
                    nc.tensor.matmul(
                        psums[m][:, poff : poff + FDP],
                        w_sb[:, g, :],
                        tg[g][:, goff : goff + FDP],
                        start=(g == 0),
                        stop=(g == GRID - 1 and last_stop),
                    )

            lo = 0
            for c, pieces in enumerate(CHUNKS):
                s = wpool.tile([128, FDE], bf16, tag="s", name=f"s_{c}")
                q = wpool.tile([128, FDE], f32, tag="q", name=f"q_{c}")
                tg = [
                    wpool.tile([128, FDE], bf16, tag=f"t{g}", name=f"t{g}_{c}")
                    for g in range(GRID)
                ]
                tgs[c] = tg
                psums[2 * c] = ppool.tile(
                    [128, TW], f32, tag="psum", name=f"psum_{2 * c}"
                )
                psums[2 * c + 1] = ppool.tile(
                    [128, TW], f32, tag="psum", name=f"psum_{2 * c + 1}"
                )
                off = 0
                mm_lo = 0
                for wd in pieces:
                    hs = slice(off, off + wd)
                    xc = x_all[:, lo + off : lo + off + wd]
                    order(
                        "s",
                        nc.scalar.activation(s[:, hs], xc, AF.Exp, scale=2.0 / W),
                    )
                    order(
                        "s",
                        nc.scalar.activation(
                            q[:, hs], xc, AF.Square, bias=1.0 / W, scale=1.0 / W
                        ),
                    )
                    order(
                        "s",
                        nc.scalar.activation(
                            tg[0][:, hs], q[:, hs], AF.Exp, scale=-1.0
                        ),
                    )
                    for g in range(1, GRID):
                        order(
                            "v",
                            nc.vector.tensor_mul(
                                tg[g][:, hs], tg[g - 1][:, hs], s[:, hs]
                            ),
                        )
                    off += wd
                    if c < 3:
                        while mm_lo + FDP <= off:
                            m = (lo + mm_lo) // TW
                            emit_gauss(
                                m, (lo + mm_lo) % TW, tg, mm_lo, m < G1
                            )
                            mm_lo += FDP
                lo += FDE
                # early drains of the g7-closed tiles, placed in the scalar
                # stream late enough to be data-ready (no queue stall) but
                # before their psum slot is needed again (M_{k+4}'s birth).
                if c == 1:
                    emit_copy(0, "s")
                if c == 2:
                    emit_copy(1, "s")
                    emit_copy(2, "s")
                if c == 3:
                    emit_copy(3, "s")

            # ---- silu phase: one table switch.  Act order: M4/M5 cols
            # (closes recycle slots 1/2 for P0/P1), then P0/P1 cols, then
            # M6/M7 cols, then P2/P3 cols.  The M4/M5 drain copies sit
            # between act0 and act1 on the scalar stream.
            silu_sb = cpool.tile([128, BLOC], bf16, name="silu_sb")

            def silu_act(k0):
                ks = slice(k0, k0 + 2048)
                order(
                    "s",
                    nc.scalar.activation(silu_sb[:, ks], x_all[:, ks], AF.Silu),
                )

            def silu_mm(ps, poff, kl, start):
                nc.tensor.matmul(
                    ps[:, poff : poff + FDP],
                    w_sb[:, 8, :],
                    silu_sb[:, kl : kl + FDP],
                    start=start,
                    stop=True,
                )

            def late_copy(ps, engine, dram, col):
                ob = opool.tile([128, TW], bf16, tag="ob", name=f"lob_{col}")
                if engine == "s":
                    order("s", nc.scalar.copy(ob[:], ps[:]))
                else:
                    order("v", nc.vector.tensor_copy(ob[:], ps[:]))
                nc.sync.dma_start(dram[:, col : col + TW], ob[:])

            # NOTE: emission order IS dependency order for psum readers — a
            # copy emitted before the silu MM would read a gauss-only sum
            # (the later MM becomes a dead WAR write).  Close M4/M5 first.
            silu_act(4096)
            for m in (4, 5):
                for sub in range(2):
                    silu_mm(psums[m], sub * FDP, m * TW + sub * FDP, False)
            late_copy(psums[4], "s", out, 4 * TW)
            late_copy(psums[5], "s", out, 5 * TW)
            silu_act(0)
            silu_act(6144)
            silu_act(2048)

            tg3 = tgs[3]
            for mi, m in enumerate((6, 7)):
                for sub in range(2):
                    goff = (mi * TW) + sub * FDP
                    for g in range(GRID - 1):
                        nc.tensor.matmul(
                            psums[m][:, sub * FDP : sub * FDP + FDP],
                            w_sb[:, g, :],
                            tg3[g][:, goff : goff + FDP],
                            start=(g == 0),
                            stop=False,
                        )

            # partial tiles P0/P1 (slots freed by the M4/M5 drains)
            pss = []
            for i in range(G1):
                pss.append(
                    ppool.tile([128, TW], f32, tag="psum", name=f"psilu_{i}")
                )
            for i in (0, 1):
                for sub in range(2):
                    silu_mm(pss[i], sub * FDP, i * TW + sub * FDP, True)

            # chain-gated: g7s + silu closes for M6/M7
            for mi, m in enumerate((6, 7)):
                for sub in range(2):
                    goff = (mi * TW) + sub * FDP
                    nc.tensor.matmul(
                        psums[m][:, sub * FDP : sub * FDP + FDP],
                        w_sb[:, 7, :],
                        tg3[7][:, goff : goff + FDP],
                        start=False,
                        stop=False,
                    )
                for sub in range(2):
                    silu_mm(psums[m], sub * FDP, m * TW + sub * FDP, False)

            # last partials (slots freed by the M6/M7 drains)
            for i in (2, 3):
                for sub in range(2):
                    silu_mm(pss[i], sub * FDP, i * TW + sub * FDP, True)

            # late drains: vector takes over once its chain ends; scalar
            # finishes with M7 and P3.  Late DMAs ride the idle sync ring.
            late_copy(pss[0], "v", outs, 0)
            late_copy(pss[1], "v", outs, 1 * TW)
            late_copy(psums[6], "v", out, 6 * TW)
            late_copy(pss[2], "v", outs, 2 * TW)
            late_copy(psums[7], "s", out, 7 * TW)
            late_copy(pss[3], "s", outs, 3 * TW)

    nc.compile()
    return nc


def _prep_weights(coeffs, base_w):
    import ml_dtypes

    g = np.arange(GRID, dtype=np.float64)
    K = np.exp(7.0 * g - g * g)  # t_g = basis_g * e^(g^2-7g) -> fold inverse
    blocks = [
        (coeffs[:, :, gi].astype(np.float64) * K[gi]).T for gi in range(GRID)
    ]  # [in, out] each
    blocks.append(base_w.astype(np.float64).T)
    wtm = np.concatenate(blocks, axis=1)  # [128, 9*128]
    return np.ascontiguousarray(wtm.astype(ml_dtypes.bfloat16))


def _gather(results):
    """Merge per-core outputs: out + silu partial for the first G1 tiles."""
    cols = []
    for c in range(NCORES):
        full = results[c]["out"].astype(np.float32)  # [128, BLOC]
        part = results[c]["outs"].astype(np.float32)  # [128, G1*TW]
        full[:, : G1 * TW] += part
        cols.append(full)
    return np.ascontiguousarray(np.concatenate(cols, axis=1).T)


def kernel(x, coeffs, base_w, centers):
    from concourse.bass_utils import run_bass_kernel_spmd

    global _NC
    if _NC is None:
        _NC = _build()

    wtm = _prep_weights(coeffs, base_w)
    xT = np.ascontiguousarray(np.asarray(x, dtype=np.float32).T)  # [128, B]
    in_maps = [
        {
            "xt": np.ascontiguousarray(xT[:, c * BLOC : (c + 1) * BLOC]),
            "wt": wtm,
        }
        for c in range(NCORES)
    ]
    res = run_bass_kernel_spmd(_NC, in_maps, list(range(NCORES)))
    return _gather(res.results)


# revision 32
# speedup vs baseline: 1.4989x; 1.0414x over previous
"""AdaptiveGridKANLayer on 8 TRN2 NeuronCores.

out[b,o] = sum_i sum_g exp(-((x[b,i]-c_g)/w)^2) * coeffs[o,i,g]
         + sum_i silu(x[b,i]) * base_w[o,i]

B=65536, in=out=128, G=8, centers = linspace(-1,1,8), w = 2/7.

Strategy (data-parallel over batch, weights replicated):
- Host: transpose x to feature-major [128, B], shard columns 8 ways; fold the
  Gaussian factorization constants e^(7g-g^2) into the coeffs.
- Device, per core (u = (x+1)/w): basis_g = e^(-(u-g)^2) = p * s^g * const
  with p = exp(-u^2) (ScalarE Square+Exp), s = exp(7x) (ScalarE Exp).
  VectorE builds the power chain t_g = t_{g-1} * s (bf16 2x-mode);
  TensorE contracts tile-major (g inner) per 512-col accumulation group.
- PSUM: 8 banks = 4 rotating slots of 1024-col f32 tiles. Main tiles
  M0..M7; M0..M3 (cols 0..4095) close at g=7 (early, chain-paced) so their
  slots recycle for M4..M7; M4..M7 are closed late by their silu matmuls
  (silu activations exist only after the one exp->silu table switch).
  M0..M3's silu contribution runs afterwards as single-MM groups P0..P3 in
  recycled slots, drained to a separate partial "outs" that the host adds
  during the unshard.
- ScalarE stream order (enforced): exp acts chunk 0..3 with the early M0..M3
  drain copies placed right after later chunks' acts (inside real pacing
  slack, never blocking the chain feed), one table switch, silu acts
  (gen-2 cols first), then its share of late drain copies.  VectorE: chain
  only, then late copies.  All engine op order is pinned (sync=False deps);
  tensor stays tile-major (g-major provokes an SBUF producer-consumer
  conflict that slows DVE/ACT ~20%).
"""

import numpy as np

BATCH = 65536
GRID = 8
NCORES = 8
BLOC = BATCH // NCORES  # 8192 batch columns per core
FDP = 512  # matmul free dim / accumulation group width
TW = 1024  # psum tile width (2 banks); 2 groups per tile
NTILE = BLOC // TW  # 8 main psum tiles
G1 = 4  # main tiles 0..3 close early at g=7; 4..7 close via silu MM
W = 2.0 / (GRID - 1)

FDE = 2048
CHUNKS = [[512, 1536], [2048], [2048], [2048]]

_NC = None


def _build():
    import concourse.mybir as mybir
    from concourse import bacc
    from concourse.tile import TileContext, add_dep_helper

    AF = mybir.ActivationFunctionType
    bf16 = mybir.dt.bfloat16
    f32 = mybir.dt.float32

    nc = bacc.Bacc("TRN2", num_devices=NCORES)
    # Bias constant for the Square activation. Must be a raw (non-pool)
    # tensor: const_aps captures the AP before pool relocation. vector
    # memset, NOT gpsimd (a single gpsimd op drags a ~6us Q7 library load
    # into the preamble).
    cst = nc.alloc_sbuf_tensor("const-float32-bias-c", [128, 1], f32)
    nc.vector.memset(cst.ap(), 1.0 / W)
    nc.const_aps.aps[(f32, 1.0 / W)] = cst.ap()
    nc.all_engine_barrier()
    xt = nc.dram_tensor("xt", [128, BLOC], f32, kind="ExternalInput").ap()
    wt = nc.dram_tensor("wt", [128, 9 * 128], bf16, kind="ExternalInput").ap()
    out = nc.dram_tensor("out", [128, BLOC], bf16, kind="ExternalOutput").ap()
    outs = nc.dram_tensor(
        "outs", [128, G1 * TW], bf16, kind="ExternalOutput"
    ).ap()

    with TileContext(nc) as tc:
        with (
            tc.tile_pool(name="const", bufs=1) as cpool,
            tc.tile_pool(name="work", bufs=3) as wpool,
            tc.tile_pool(name="obuf", bufs=6) as opool,
            tc.tile_pool(name="psum", bufs=4, space="PSUM") as ppool,
        ):
            # Exp table load during preamble (no DMA deps).
            warm_act = cpool.tile([128, 1], f32, name="warm_act")
            nc.vector.memset(warm_act[:], 0.0)
            nc.scalar.activation(warm_act[:], warm_act[:], AF.Exp, scale=1.0)

            # PE HAM clock warm: memset-fed matmuls (no DMA deps).
            wm_s = cpool.tile([128, 128], bf16, name="wm_s")
            wm_m = cpool.tile([128, 256], bf16, name="wm_m")
            nc.vector.memset(wm_s[:], 0.25)
            nc.vector.memset(wm_m[:], 0.25)
            warm_ps = ppool.tile([128, 256], f32, name="warm_ps", tag="psum")
            for _ in range(10):
                nc.tensor.matmul(
                    warm_ps[:], wm_s[:], wm_m[:], start=True, stop=True
                )

            # x stream alternating across BOTH HWDGE rings (sync + scalar):
            # a single ring serializes the pieces and paces the whole left
            # half of the pipeline.  Weights ride the scalar ring early.
            w_sb = cpool.tile([128, 9, 128], bf16, name="w_sb")
            x_all = cpool.tile([128, BLOC], f32, name="x_all")
            lo = 0
            for i, wd in enumerate([p for ch in CHUNKS for p in ch]):
                eng = nc.sync if i % 2 == 0 else nc.scalar
                eng.dma_start(x_all[:, lo : lo + wd], xt[:, lo : lo + wd])
                lo += wd
                if i == 2:
                    nc.scalar.dma_start(
                        w_sb[:], wt.rearrange("p (g o) -> p g o", g=9)
                    )

            psums = [None] * NTILE
            prev_op = {"s": None, "v": None}

            def order(eng, op):
                if prev_op[eng] is not None:
                    add_dep_helper(op.ins, prev_op[eng].ins, False, "order")
                prev_op[eng] = op
                return op

            def emit_copy(m, engine, ordered=True):
                ob = opool.tile([128, TW], bf16, tag="ob", name=f"ob_{m}")
                if engine == "s":
                    op = nc.scalar.copy(ob[:], psums[m][:])
                else:
                    op = nc.vector.tensor_copy(ob[:], psums[m][:])
                if ordered:
                    order(engine, op)
                nc.scalar.dma_start(out[:, m * TW : (m + 1) * TW], ob[:])

            # ---- exp phase: scalar acts, vector chain, tensor tile-major.
            # Gauss matmuls are emitted per completed 512-col block (pieces
            # can be narrower than a block).  Chunk 3's gauss matmuls are
            # deferred so the tensor tail can interleave silu/partial work.
            tgs = [None] * len(CHUNKS)

            def emit_gauss(m, poff, tg, goff, last_stop):
                for g in range(GRID):
                    nc.tensor.matmul(
                        psums[m][:, poff : poff + FDP],
                        w_sb[:, g, :],
                        tg[g][:, goff : goff + FDP],
                        start=(g == 0),
                        stop=(g == GRID - 1 and last_stop),
                    )

            lo = 0
            for c, pieces in enumerate(CHUNKS):
                s = wpool.tile([128, FDE], bf16, tag="s", name=f"s_{c}")
                q = wpool.tile([128, FDE], f32, tag="q", name=f"q_{c}")
                tg = [
                    wpool.tile([128, FDE], bf16, tag=f"t{g}", name=f"t{g}_{c}")
                    for g in range(GRID)
                ]
                tgs[c] = tg
                psums[2 * c] = ppool.tile(
                    [128, TW], f32, tag="psum", name=f"psum_{2 * c}"
                )
                psums[2 * c + 1] = ppool.tile(
                    [128, TW], f32, tag="psum", name=f"psum_{2 * c + 1}"
                )
                off = 0
                mm_lo = 0
                for wd in pieces:
                    hs = slice(off, off + wd)
                    xc = x_all[:, lo + off : lo + off + wd]
                    order(
                        "s",
                        nc.scalar.activation(s[:, hs], xc, AF.Exp, scale=2.0 / W),
                    )
                    order(
                        "s",
                        nc.scalar.activation(
                            q[:, hs], xc, AF.Square, bias=1.0 / W, scale=1.0 / W
                        ),
                    )
                    order(
                        "s",
                        nc.scalar.activation(
                            tg[0][:, hs], q[:, hs], AF.Exp, scale=-1.0
                        ),
                    )
                    for g in range(1, GRID):
                        order(
                            "v",
                            nc.vector.tensor_mul(
                                tg[g][:, hs], tg[g - 1][:, hs], s[:, hs]
                            ),
                        )
                    off += wd
                    if c < 3:
                        while mm_lo + FDP <= off:
                            m = (lo + mm_lo) // TW
                            emit_gauss(
                                m, (lo + mm_lo) % TW, tg, mm_lo, m < G1
                            )
                            mm_lo += FDP
                lo += FDE
                # early drains of the g7-closed tiles, placed in the scalar
                # stream late enough to be data-ready (no queue stall) but
                # before their psum slot is needed again (M_{k+4}'s birth).
                if c == 2:
                    emit_copy(0, "s")
                    emit_copy(1, "s")
                if c == 3:
                    emit_copy(2, "s")
                    emit_copy(3, "s")

            # ---- silu phase: one table switch.  Act order: M4/M5 cols
            # (closes recycle slots 1/2 for P0/P1), then P0/P1 cols, then
            # M6/M7 cols, then P2/P3 cols.  The M4/M5 drain copies sit
            # between act0 and act1 on the scalar stream.
            silu_sb = cpool.tile([128, BLOC], bf16, name="silu_sb")

            def silu_act(k0):
                ks = slice(k0, k0 + 2048)
                order(
                    "s",
                    nc.scalar.activation(silu_sb[:, ks], x_all[:, ks], AF.Silu),
                )

            def silu_mm(ps, poff, kl, start):
                nc.tensor.matmul(
                    ps[:, poff : poff + FDP],
                    w_sb[:, 8, :],
                    silu_sb[:, kl : kl + FDP],
                    start=start,
                    stop=True,
                )

            ndma = [0]

            def late_copy(ps, engine, dram, col):
                ob = opool.tile([128, TW], bf16, tag="ob", name=f"lob_{col}")
                if engine == "s":
                    order("s", nc.scalar.copy(ob[:], ps[:]))
                else:
                    order("v", nc.vector.tensor_copy(ob[:], ps[:]))
                deng = nc.sync if ndma[0] % 2 == 0 else nc.scalar
                ndma[0] += 1
                deng.dma_start(dram[:, col : col + TW], ob[:])

            # NOTE: emission order IS dependency order for psum readers — a
            # copy emitted before the silu MM would read a gauss-only sum
            # (the later MM becomes a dead WAR write).  Close M4/M5 first.
            silu_act(4096)
            for m in (4, 5):
                for sub in range(2):
                    silu_mm(psums[m], sub * FDP, m * TW + sub * FDP, False)
            late_copy(psums[4], "s", out, 4 * TW)
            late_copy(psums[5], "s", out, 5 * TW)
            silu_act(6144)
            silu_act(0)
            silu_act(2048)

            tg3 = tgs[3]
            for mi, m in enumerate((6, 7)):
                for sub in range(2):
                    goff = (mi * TW) + sub * FDP
                    for g in range(GRID - 1):
                        nc.tensor.matmul(
                            psums[m][:, sub * FDP : sub * FDP + FDP],
                            w_sb[:, g, :],
                            tg3[g][:, goff : goff + FDP],
                            start=(g == 0),
                            stop=False,
                        )

            # partial tiles P0/P1 (slots freed by the M4/M5 drains)
            pss = []
            for i in range(G1):
                pss.append(
                    ppool.tile([128, TW], f32, tag="psum", name=f"psilu_{i}")
                )
            for i in (0, 1):
                for sub in range(2):
                    silu_mm(pss[i], sub * FDP, i * TW + sub * FDP, True)

            # chain-gated: g7s + silu closes for M6/M7
            for mi, m in enumerate((6, 7)):
                for sub in range(2):
                    goff = (mi * TW) + sub * FDP
                    nc.tensor.matmul(
                        psums[m][:, sub * FDP : sub * FDP + FDP],
                        w_sb[:, 7, :],
                        tg3[7][:, goff : goff + FDP],
                        start=False,
                        stop=False,
                    )
                for sub in range(2):
                    silu_mm(psums[m], sub * FDP, m * TW + sub * FDP, False)

            # last partials (slots freed by the M6/M7 drains)
            for i in (2, 3):
                for sub in range(2):
                    silu_mm(pss[i], sub * FDP, i * TW + sub * FDP, True)

            # late drains: vector takes over once its chain ends; scalar
            # finishes with M7 and P3.  DMAs alternate the idle rings.
            late_copy(psums[6], "v", out, 6 * TW)
            late_copy(pss[0], "v", outs, 0)
            late_copy(pss[1], "v", outs, 1 * TW)
            late_copy(pss[2], "v", outs, 2 * TW)
            late_copy(psums[7], "s", out, 7 * TW)
            late_copy(pss[3], "s", outs, 3 * TW)

    nc.compile()
    return nc


def _prep_weights(coeffs, base_w):
    import ml_dtypes

    g = np.arange(GRID, dtype=np.float64)
    K = np.exp(7.0 * g - g * g)  # t_g = basis_g * e^(g^2-7g) -> fold inverse
    blocks = [
        (coeffs[:, :, gi].astype(np.float64) * K[gi]).T for gi in range(GRID)
    ]  # [in, out] each
    blocks.append(base_w.astype(np.float64).T)
    wtm = np.concatenate(blocks, axis=1)  # [128, 9*128]
    return np.ascontiguousarray(wtm.astype(ml_dtypes.bfloat16))


def _gather(results):
    """Merge per-core outputs: out + silu partial for the first G1 tiles."""
    cols = []
    for c in range(NCORES):
        full = results[c]["out"].astype(np.float32)  # [128, BLOC]
        part = results[c]["outs"].astype(np.float32)  # [128, G1*TW]
        full[:, : G1 * TW] += part
        cols.append(full)
    return np.ascontiguousarray(np.concatenate(cols, axis=1).T)


def kernel(x, coeffs, base_w, centers):
    from concourse.bass_utils import run_bass_kernel_spmd

    global _NC
    if _NC is None:
        _NC = _build()

    wtm = _prep_weights(coeffs, base_w)
    xT = np.ascontiguousarray(np.asarray(x, dtype=np.float32).T)  # [128, B]
    in_maps = [
        {
            "xt": np.ascontiguousarray(xT[:, c * BLOC : (c + 1) * BLOC]),
            "wt": wtm,
        }
        for c in range(NCORES)
    ]
    res = run_bass_kernel_spmd(_NC, in_maps, list(range(NCORES)))
    return _gather(res.results)
